# revision 1
# baseline (speedup 1.0000x reference)
"""HGNN conv kernel for 8 Trainium2 NeuronCores.

Computes out = segment_sum(g_vals * (x @ W + b)[g_cols], g_rows, N)
reordered as out = (G @ x) @ W + rowsum(G) outer b, so that no
cross-core communication is needed: destination rows are sharded
across the 8 cores, x is replicated into every core's DRAM, and each
core gathers the source rows it needs with SWDGE dma_gather.

Per core (12500 dest rows = 98 tiles of 128):
  stage 1 (SpMM): for each dest tile, gather the tile's source rows
    (sorted by dest, grouped into 4 source-index windows so the int16
    gather indices fit), build a one-hot-times-val matrix A on the DVE
    (iota == dest compare, then * val), and accumulate
    psum_S = sum_k A_k^T @ R_k on the PE (float32r: 1 cycle/row).
  stage 2 (GEMM): PE-transpose S, then out = S @ W + rowsum x b via
    4 chunked matmuls plus a K=1 bias matmul, all accumulated in PSUM.
"""

import os
import sys

import numpy as np

sys.path.insert(0, "/opt/trn_rl_repo")

import concourse.bacc as bacc
import concourse.bass as bass
import concourse.mybir as mybir
import concourse.tile as tile
from concourse.bass_utils import run_bass_kernel_spmd


def _install_ntff_hook():
    """The agent image's antenv lacks axon_hooks; synthesize it so
    run_bass_kernel_spmd(trace=True) can capture NTFF profiles."""
    import types
    if "antenv.axon_hooks" in sys.modules:
        return
    mod = types.ModuleType("antenv.axon_hooks")
    _h = [None]
    mod.set_axon_ntff_profile_hook = lambda h: _h.__setitem__(0, h)
    mod.get_axon_ntff_profile_hook = lambda: _h[0]
    sys.modules["antenv.axon_hooks"] = mod
    import antenv
    antenv.axon_hooks = mod
    from trn_agent_boot.trn_boot import _ntff_profile_via_ctypes
    mod.set_axon_ntff_profile_hook(
        _ntff_profile_via_ctypes("/opt/axon/libaxon_pjrt.so")
    )


_install_ntff_hook()

N = 100000
F = 512
CORES = 8
RPC = 12500            # dest rows per core
TILES = 98             # ceil(12500 / 128)
NPAD = TILES * 128     # 12544
SRC_CHUNK = 25000
GROUPS = 4
GW = SRC_CHUNK + 1     # group window rows incl. one zero pad row
XROWS = GROUPS * GW    # 100004
PAD_LOCAL = SRC_CHUNK  # local index of the zero pad row in each window

F32 = mybir.dt.float32
F32R = mybir.dt.float32r
BF16 = mybir.dt.bfloat16
I16 = mybir.dt.int16
MMDT = BF16            # matmul dtype for the SpMM/GEMM data path
import ml_dtypes
NPDT = ml_dtypes.bfloat16


def _preprocess(x, g_rows, g_cols, g_vals):
    """Sort/pad edges into the per-core, per-tile, per-group chunk layout."""
    rows = np.asarray(g_rows, dtype=np.int64)
    cols = np.asarray(g_cols, dtype=np.int64)
    vals = np.asarray(g_vals, dtype=np.float32)

    core = rows // RPC
    rl = rows - core * RPC          # 0..12499 local dest row
    tile_i = rl >> 7
    grp = cols // SRC_CHUNK
    sloc = (cols - grp * SRC_CHUNK).astype(np.int16)

    key = ((core * TILES + tile_i) * GROUPS + grp) * SRC_CHUNK + (cols - grp * SRC_CHUNK)
    order = np.argsort(key, kind="stable")

    bucket = (core * TILES + tile_i) * GROUPS + grp
    cnt = np.bincount(bucket, minlength=CORES * TILES * GROUPS).reshape(
        CORES, TILES * GROUPS
    )
    # cross-core-uniform chunk counts per (tile, group)
    n_chunks = -(-cnt.max(axis=0) // 128)            # [TILES*GROUPS]
    TC = int(n_chunks.sum())
    col_off = np.zeros(TILES * GROUPS + 1, np.int64)
    np.cumsum(n_chunks, out=col_off[1:])
    slot_off = col_off * 128
    SLOTS = TC * 128

    core_cnt = np.bincount(core, minlength=CORES)
    core_start = np.zeros(CORES + 1, np.int64)
    np.cumsum(core_cnt, out=core_start[1:])

    gidx = np.empty((CORES, 128, TC * 8), np.int16)
    gdst = np.empty((CORES, 128, TC), np.float32)
    gval = np.empty((CORES, 128, TC), np.float32)
    rsum = np.zeros((CORES, NPAD), np.float32)

    nch = n_chunks  # flat [TILES*GROUPS]
    for c in range(CORES):
        seg = order[core_start[c]:core_start[c + 1]]
        tg = tile_i[seg] * GROUPS + grp[seg]         # non-decreasing
        cnt_tg = np.bincount(tg, minlength=TILES * GROUPS)
        gstart = np.zeros(TILES * GROUPS, np.int64)
        np.cumsum(cnt_tg[:-1], out=gstart[1:])
        pos = np.arange(len(seg), dtype=np.int64) - np.repeat(gstart, cnt_tg)
        slot = slot_off[tg] + pos

        idx_flat = np.full(SLOTS, PAD_LOCAL, np.int16)
        idx_flat[slot] = sloc[seg]
        d_flat = np.zeros(SLOTS, np.float32)
        d_flat[slot] = (rl[seg] & 127).astype(np.float32)
        v_flat = np.zeros(SLOTS, np.float32)
        v_flat[slot] = vals[seg]

        gdst[c] = d_flat.reshape(TC, 128).T
        gval[c] = v_flat.reshape(TC, 128).T
        # idx wrap: within each (t,g) call, idx j -> [j%16, j//16], x8 replicated
        for tg_i in range(TILES * GROUPS):
            n = nch[tg_i]
            if n == 0:
                continue
            a = slot_off[tg_i]
            bcol = col_off[tg_i] * 8
            blk = idx_flat[a:a + n * 128].reshape(n * 8, 16).T
            gidx[c][:, bcol:bcol + n * 8] = np.tile(blk, (8, 1))

        rs = np.bincount(rl[seg], weights=vals[seg].astype(np.float64),
                         minlength=RPC)
        rsum[c][:RPC] = rs.astype(np.float32)

    return (n_chunks.reshape(TILES, GROUPS), TC, gidx, gdst, gval,
            rsum.reshape(CORES, TILES, 128))


def _build_program(n_chunks, TC):
    nch = n_chunks  # [TILES, GROUPS]
    GMAX = int(nch.max())
    TMAX = int(nch.sum(axis=1).max())

    nc = bacc.Bacc(
        "TRN2",
        target_bir_lowering=False,
        debug=False,
        enable_asserts=False,
        num_devices=CORES,
        num_swdge_queues=4,
    )
    xdev = nc.dram_tensor("xdev", [XROWS, F], MMDT, kind="ExternalInput").ap()
    gidx = nc.dram_tensor("gidx", [128, TC * 8], I16, kind="ExternalInput").ap()
    gdst = nc.dram_tensor("gdst", [128, TC], F32, kind="ExternalInput").ap()
    gval = nc.dram_tensor("gval", [128, TC], MMDT, kind="ExternalInput").ap()
    wmat = nc.dram_tensor("wmat", [F, F], MMDT, kind="ExternalInput").ap()
    bvec = nc.dram_tensor("bvec", [1, F], MMDT, kind="ExternalInput").ap()
    rsum = nc.dram_tensor("rsum", [TILES, 128], MMDT, kind="ExternalInput").ap()
    iot = nc.dram_tensor("iot", [128, 128], F32, kind="ExternalInput").ap()
    identt = nc.dram_tensor("identt", [128, 128], F32, kind="ExternalInput").ap()
    out = nc.dram_tensor("out", [NPAD, F], F32, kind="ExternalOutput").ap()

    from contextlib import ExitStack

    with tile.TileContext(nc) as tc, ExitStack() as ctx:
        cpool = ctx.enter_context(tc.tile_pool(name="const", bufs=1))
        idxp = ctx.enter_context(tc.tile_pool(name="idxp", bufs=6))
        dvp = ctx.enter_context(tc.tile_pool(name="dvp", bufs=3))
        rpool = ctx.enter_context(tc.tile_pool(name="rp", bufs=3))
        apool = ctx.enter_context(tc.tile_pool(name="ap", bufs=2))
        spool = ctx.enter_context(tc.tile_pool(name="sp", bufs=2))
        opool = ctx.enter_context(tc.tile_pool(name="op", bufs=2))
        psS = ctx.enter_context(tc.tile_pool(name="psS", bufs=2, space="PSUM"))
        psT = ctx.enter_context(tc.tile_pool(name="psT", bufs=2, space="PSUM"))
        psO = ctx.enter_context(tc.tile_pool(name="psO", bufs=2, space="PSUM"))

        w_t = cpool.tile([128, 4, F], MMDT)
        for k in range(4):
            nc.sync.dma_start(w_t[:, k, :], wmat[k * 128:(k + 1) * 128, :])
        b_t = cpool.tile([1, F], MMDT)
        nc.sync.dma_start(b_t[:], bvec[:])
        io_t = cpool.tile([128, 128], F32)
        nc.sync.dma_start(io_t[:], iot[:])
        id_t = cpool.tile([128, 128], F32)
        nc.sync.dma_start(id_t[:], identt[:])

        qn = 0
        c0 = 0
        for t in range(TILES):
            tc_t = int(nch[t].sum())
            pS = psS.tile([128, F], F32)
            rs_t = dvp.tile([1, 128], MMDT, tag="rs")
            nc.sync.dma_start(rs_t[:], rsum[t:t + 1, :])
            dst_t = dvp.tile([128, TMAX], F32, tag="dst")
            nc.sync.dma_start(dst_t[:, :tc_t], gdst[:, c0:c0 + tc_t])
            val_t = dvp.tile([128, TMAX], MMDT, tag="val")
            nc.sync.dma_start(val_t[:, :tc_t], gval[:, c0:c0 + tc_t])
            A = apool.tile([128, TMAX, 128], MMDT)
            nc.vector.tensor_tensor(
                out=A[:, :tc_t, :],
                in0=io_t[:].unsqueeze(1).to_broadcast([128, tc_t, 128]),
                in1=dst_t[:, :tc_t].unsqueeze(2).to_broadcast([128, tc_t, 128]),
                op=mybir.AluOpType.is_equal,
            )
            nc.vector.tensor_tensor(
                out=A[:, :tc_t, :],
                in0=A[:, :tc_t, :],
                in1=val_t[:, :tc_t].unsqueeze(2).to_broadcast([128, tc_t, 128]),
                op=mybir.AluOpType.mult,
            )
            kk = 0
            for g in range(GROUPS):
                n = int(nch[t][g])
                if n == 0:
                    continue
                it = idxp.tile([128, max(int(nch.max()), 1) * 8], I16)
                nc.sync.dma_start(
                    it[:, :n * 8], gidx[:, (c0 + kk) * 8:(c0 + kk + n) * 8]
                )
                R = rpool.tile([128, max(int(nch.max()), 1), F], MMDT)
                # ucode caps one dma_gather at 1024 indices (8 chunks)
                for b0 in range(0, n, 8):
                    nb = min(8, n - b0)
                    nc.gpsimd.dma_gather(
                        out_ap=R[:, b0:b0 + nb, :],
                        in_ap=xdev[g * GW:(g + 1) * GW, :],
                        idxs_ap=it[:, b0 * 8:(b0 + nb) * 8],
                        num_idxs=nb * 128,
                        num_idxs_reg=nb * 128,
                        elem_size=F,
                        queue_num=qn,
                    )
                    qn = (qn + 1) % 4
                for k in range(n):
                    nc.tensor.matmul(
                        pS[:],
                        lhsT=A[:, kk + k, :],
                        rhs=R[:, k, :],
                        start=(kk + k == 0),
                        stop=(kk + k == tc_t - 1),
                    )
                kk += n

            S = spool.tile([128, F], F32)
            nc.vector.tensor_copy(S[:], pS[:])
            pT = psT.tile([128, F], F32)
            for k in range(4):
                nc.tensor.transpose(
                    pT[:, k * 128:(k + 1) * 128], S[:, k * 128:(k + 1) * 128], id_t[:]
                )
            ST = spool.tile([128, F], MMDT)
            nc.vector.tensor_copy(ST[:], pT[:])
            pO = psO.tile([128, F], F32)
            for k in range(4):
                nc.tensor.matmul(
                    pO[:],
                    lhsT=ST[:, k * 128:(k + 1) * 128],
                    rhs=w_t[:, k, :],
                    start=(k == 0),
                    stop=False,
                )
            nc.tensor.matmul(
                pO[:],
                lhsT=rs_t[0:1, :],
                rhs=b_t[0:1, :],
                start=False,
                stop=True,
            )
            O = opool.tile([128, F], F32)
            nc.vector.tensor_copy(O[:], pO[:])
            nc.sync.dma_start(out[t * 128:(t + 1) * 128, :], O[:])
            c0 += tc_t

    nc.compile()
    return nc


def kernel(x, g_rows, g_cols, g_vals, weight, b, trace=False):
    x = np.asarray(x, dtype=np.float32)
    weight = np.asarray(weight, dtype=np.float32)
    b = np.asarray(b, dtype=np.float32)

    n_chunks, TC, gidx, gdst, gval, rsum = _preprocess(x, g_rows, g_cols, g_vals)
    TMAX = int(n_chunks.sum(axis=1).max())

    x_dev = np.zeros((XROWS, F), NPDT)
    for g in range(GROUPS):
        x_dev[g * GW:g * GW + SRC_CHUNK] = x[g * SRC_CHUNK:(g + 1) * SRC_CHUNK]
    iota2 = np.broadcast_to(
        np.arange(128, dtype=np.float32)[None, :], (128, 128)
    ).copy()
    ident = np.eye(128, dtype=np.float32)

    nc = _build_program(n_chunks, TC)

    in_maps = []
    for c in range(CORES):
        in_maps.append({
            "xdev": x_dev,
            "gidx": gidx[c],
            "gdst": gdst[c],
            "gval": gval[c].astype(NPDT),
            "wmat": weight.astype(NPDT),
            "bvec": b.reshape(1, F).astype(NPDT),
            "rsum": rsum[c].astype(NPDT),
            "iot": iota2,
            "identt": ident,
        })

    res = run_bass_kernel_spmd(nc, in_maps, core_ids=list(range(CORES)), trace=trace)
    outs = [res.results[c]["out"][:RPC] for c in range(CORES)]
    full = np.concatenate(outs, axis=0)
    kernel.last_exec_time_ns = res.exec_time_ns
    kernel.last_results = res
    return full



# revision 3
# speedup vs baseline: 1.5953x; 1.5953x over previous
"""HGNN conv kernel for 8 Trainium2 NeuronCores.

Computes out = segment_sum(g_vals * (x @ W + b)[g_cols], g_rows, N)
reordered as out = (G @ x) @ W + rowsum(G) outer b, so that no
cross-core communication is needed: destination rows are sharded
across the 8 cores, x is replicated into every core's DRAM, and each
core gathers the source rows it needs with SWDGE dma_gather.

v2 layout: dest rows are BIN-PACKED (not contiguous) into the
8*98 = 784 tiles of 128 rows so that every (tile, source-window)
bucket lands just under a uniform chunk budget (9,9,8,8 chunks of
128 edges) -> ~3% slot padding instead of ~13%, uniform 1024-index
gather calls, and cross-core-identical loop bounds for the single
SPMD program.  dst/val/rsum live in SBUF for the whole kernel (two
DMAs at startup), gathers run on 4 SWDGE queues with 10 R-buffers of
prefetch depth, A-matrix build stays on the DVE, PSUM->SBUF copies
move to the idle Activation engine, and the output is written bf16.
"""

import os
import sys

import numpy as np

sys.path.insert(0, "/opt/trn_rl_repo")

import concourse.bacc as bacc
import concourse.bass as bass
import concourse.mybir as mybir
import concourse.tile as tile
from concourse.bass_utils import run_bass_kernel_spmd


def _install_ntff_hook():
    """The agent image's antenv lacks axon_hooks; synthesize it so
    run_bass_kernel_spmd(trace=True) can capture NTFF profiles."""
    import types
    if "antenv.axon_hooks" in sys.modules:
        return
    mod = types.ModuleType("antenv.axon_hooks")
    _h = [None]
    mod.set_axon_ntff_profile_hook = lambda h: _h.__setitem__(0, h)
    mod.get_axon_ntff_profile_hook = lambda: _h[0]
    sys.modules["antenv.axon_hooks"] = mod
    import antenv
    antenv.axon_hooks = mod
    from trn_agent_boot.trn_boot import _ntff_profile_via_ctypes
    mod.set_axon_ntff_profile_hook(
        _ntff_profile_via_ctypes("/opt/axon/libaxon_pjrt.so")
    )


_install_ntff_hook()

N = 100000
F = 512
CORES = 8
TILES = 98
NPAD = TILES * 128     # 12544 output rows per core
NBINS = CORES * TILES  # 784
GROUPS = 4
# source-column windows sized proportional to the (9,9,8,8) chunk
# budget so every bucket has ~6.6% slack vs its cap
GB = np.array([0, 26471, 52942, 76471, 100000], dtype=np.int64)
WSZ = np.diff(GB)                       # [26471, 26471, 23529, 23529]
WBASE = np.concatenate([[0], np.cumsum(WSZ + 1)])  # window bases in xdev
XROWS = int(WBASE[-1])                  # 100004 (one zero pad row per window)
CAPS = np.array([9 * 128, 9 * 128, 8 * 128, 8 * 128], dtype=np.int64)

F32 = mybir.dt.float32
BF16 = mybir.dt.bfloat16
I16 = mybir.dt.int16
MMDT = BF16
import ml_dtypes
NPDT = ml_dtypes.bfloat16


def _pack_rows(D):
    """Assign each dest row to one of 784 bins (<=128 rows each) so that
    per-bin per-group edge counts stay under CAPS.  Snake assignment by
    total degree, then local swap repair."""
    totdeg = D.sum(axis=1)
    order = np.argsort(-totdeg, kind="stable")
    i = np.arange(N)
    r, p = i // NBINS, i % NBINS
    binid = np.where(r % 2 == 0, p, NBINS - 1 - p)
    assign = np.empty(N, np.int64)
    assign[order] = binid

    Fb = np.zeros((NBINS, GROUPS), np.int64)
    np.add.at(Fb, assign, D)

    bins_rows = [[] for _ in range(NBINS)]
    for row in order:  # keep degree-desc order inside each bin
        bins_rows[assign[row]].append(row)

    rng = np.random.default_rng(0)
    for _ in range(4000):
        over = Fb - CAPS
        b, g = np.unravel_index(np.argmax(over), over.shape)
        if over[b, g] <= 0:
            break
        # row in b with max group-g degree
        rb = bins_rows[b]
        dgs = D[rb, g]
        r1_i = int(np.argmax(dgs))
        r1 = rb[r1_i]
        # candidate donor bins, least loaded on g first
        for b2 in np.argsort(Fb[:, g])[:16]:
            if b2 == b:
                continue
            rb2 = bins_rows[b2]
            dgs2 = D[rb2, g]
            r2_i = int(np.argmin(dgs2))
            r2 = rb2[r2_i]
            d1, d2 = D[r1], D[r2]
            if d1[g] <= d2[g]:
                continue
            nb2 = Fb[b2] + d1 - d2
            nb = Fb[b] - d1 + d2
            if (nb2 <= CAPS).all() and (nb <= np.maximum(Fb[b], CAPS)).all():
                rb[r1_i] = r2
                rb2[r2_i] = r1
                Fb[b] = nb
                Fb[b2] = nb2
                break
        else:
            # no donor found; perturb by moving on a random coordinate
            continue
    return bins_rows, Fb


def _preprocess(g_rows, g_cols, g_vals):
    rows = np.asarray(g_rows, dtype=np.int64)
    cols = np.asarray(g_cols, dtype=np.int64)
    vals = np.asarray(g_vals, dtype=np.float32)
    NNZ = rows.shape[0]

    grp = np.searchsorted(GB[1:-1], cols, side="right")
    D = np.bincount(rows * GROUPS + grp, minlength=N * GROUPS).reshape(N, GROUPS)

    bins_rows, Fb = _pack_rows(D)

    # bins -> (tile, core), grouping bins with equal chunk-need profiles
    need = -(-Fb // 128)  # [784, 4]
    binorder = np.lexsort((need[:, 3], need[:, 2], need[:, 1], need[:, 0]))
    core_of_row = np.empty(N, np.int32)
    tile_of_row = np.empty(N, np.int32)
    loc_of_row = np.empty(N, np.int32)
    rowmap = np.full((CORES, NPAD), -1, np.int64)
    n_chunks = np.zeros((TILES, GROUPS), np.int64)
    for idx, b in enumerate(binorder):
        t, c = idx // CORES, idx % CORES
        lst = bins_rows[b]
        core_of_row[lst] = c
        tile_of_row[lst] = t
        loc_of_row[lst] = np.arange(len(lst))
        rowmap[c, t * 128:t * 128 + len(lst)] = lst
        n_chunks[t] = np.maximum(n_chunks[t], need[b])
    n_chunks = np.maximum(n_chunks, 1)

    TC = int(n_chunks.sum())
    ncf = n_chunks.reshape(-1)                      # [TILES*GROUPS]
    col_off = np.zeros(TILES * GROUPS + 1, np.int64)
    np.cumsum(ncf, out=col_off[1:])
    slot_off = col_off * 128
    SLOTS = TC * 128

    ec = core_of_row[rows]
    et = tile_of_row[rows]
    el = loc_of_row[rows]
    sloc = (cols - GB[grp]).astype(np.int16)

    key = (((ec.astype(np.int64) * TILES + et) * GROUPS + grp) * (2**18)
           + cols - GB[grp])
    order = np.argsort(key, kind="stable")

    core_cnt = np.bincount(ec, minlength=CORES)
    core_start = np.zeros(CORES + 1, np.int64)
    np.cumsum(core_cnt, out=core_start[1:])

    # default pad index per slot = its window's zero pad row
    wsz_tg = np.tile(WSZ, TILES)                    # [TILES*GROUPS]
    pad_flat = np.repeat(wsz_tg, ncf * 128).astype(np.int16)

    gidx = np.empty((CORES, 128, TC * 8), np.int16)
    gdst = np.empty((CORES, 128, TC), np.float32)
    gval = np.empty((CORES, 128, TC), np.float32)
    rsum = np.zeros((CORES, NPAD), np.float32)

    for c in range(CORES):
        seg = order[core_start[c]:core_start[c + 1]]
        tg = et[seg].astype(np.int64) * GROUPS + grp[seg]   # non-decreasing
        cnt_tg = np.bincount(tg, minlength=TILES * GROUPS)
        assert (cnt_tg <= ncf * 128).all()
        gstart = np.zeros(TILES * GROUPS, np.int64)
        np.cumsum(cnt_tg[:-1], out=gstart[1:])
        pos = np.arange(len(seg), dtype=np.int64) - np.repeat(gstart, cnt_tg)
        slot = slot_off[tg] + pos

        idx_flat = pad_flat.copy()
        idx_flat[slot] = sloc[seg]
        d_flat = np.zeros(SLOTS, np.float32)
        d_flat[slot] = el[seg].astype(np.float32)
        v_flat = np.zeros(SLOTS, np.float32)
        v_flat[slot] = vals[seg]

        gdst[c] = d_flat.reshape(TC, 128).T
        gval[c] = v_flat.reshape(TC, 128).T
        # idx wrap: within each (t,g) bucket, idx j -> [j%16, j//16], x8 replicated
        for tg_i in range(TILES * GROUPS):
            n = ncf[tg_i]
            a = slot_off[tg_i]
            bcol = col_off[tg_i] * 8
            blk = idx_flat[a:a + n * 128].reshape(n * 8, 16).T
            gidx[c][:, bcol:bcol + n * 8] = np.tile(blk, (8, 1))

        rs = np.bincount(et[seg] * 128 + el[seg],
                         weights=vals[seg].astype(np.float64), minlength=NPAD)
        rsum[c] = rs.astype(np.float32)

    return n_chunks, TC, gidx, gdst, gval, rsum, rowmap


def _build_program(n_chunks, TC):
    nch = n_chunks  # [TILES, GROUPS]
    TMAX = int(nch.sum(axis=1).max())

    nc = bacc.Bacc(
        "TRN2",
        target_bir_lowering=False,
        debug=False,
        enable_asserts=False,
        num_devices=CORES,
        num_swdge_queues=4,
    )
    xdev = nc.dram_tensor("xdev", [XROWS, F], MMDT, kind="ExternalInput").ap()
    gidx = nc.dram_tensor("gidx", [128, TC * 8], I16, kind="ExternalInput").ap()
    gdst = nc.dram_tensor("gdst", [128, TC], F32, kind="ExternalInput").ap()
    gval = nc.dram_tensor("gval", [128, TC], MMDT, kind="ExternalInput").ap()
    wmat = nc.dram_tensor("wmat", [F, F], MMDT, kind="ExternalInput").ap()
    bvec = nc.dram_tensor("bvec", [1, F], MMDT, kind="ExternalInput").ap()
    rsum = nc.dram_tensor("rsum", [1, NPAD], MMDT, kind="ExternalInput").ap()
    iot = nc.dram_tensor("iot", [128, 128], F32, kind="ExternalInput").ap()
    identt = nc.dram_tensor("identt", [128, 128], MMDT, kind="ExternalInput").ap()
    out = nc.dram_tensor("out", [NPAD, F], MMDT, kind="ExternalOutput").ap()

    from contextlib import ExitStack

    with tile.TileContext(nc) as tc, ExitStack() as ctx:
        cpool = ctx.enter_context(tc.tile_pool(name="const", bufs=1))
        idxp = ctx.enter_context(tc.tile_pool(name="idxp", bufs=6))
        apool = ctx.enter_context(tc.tile_pool(name="ap", bufs=2))
        rpool = ctx.enter_context(tc.tile_pool(name="rp", bufs=10))
        spool = ctx.enter_context(tc.tile_pool(name="sp", bufs=2))
        stpool = ctx.enter_context(tc.tile_pool(name="stp", bufs=2))
        opool = ctx.enter_context(tc.tile_pool(name="op", bufs=2))
        psS = ctx.enter_context(tc.tile_pool(name="psS", bufs=2, space="PSUM"))
        psT = ctx.enter_context(tc.tile_pool(name="psT", bufs=2, space="PSUM"))
        psO = ctx.enter_context(tc.tile_pool(name="psO", bufs=2, space="PSUM"))

        # static SBUF residents
        w_t = cpool.tile([128, 4, F], MMDT)
        for k in range(4):
            nc.sync.dma_start(w_t[:, k, :], wmat[k * 128:(k + 1) * 128, :])
        b_t = cpool.tile([1, F], MMDT)
        nc.sync.dma_start(b_t[:], bvec[:])
        io_t = cpool.tile([128, 128], F32)
        nc.sync.dma_start(io_t[:], iot[:])
        id_t = cpool.tile([128, 128], MMDT)
        nc.sync.dma_start(id_t[:], identt[:])
        dst_r = cpool.tile([128, TC], F32)
        nc.sync.dma_start(dst_r[:], gdst[:])
        val_r = cpool.tile([128, TC], MMDT)
        nc.sync.dma_start(val_r[:], gval[:])
        rs_r = cpool.tile([1, NPAD], MMDT)
        nc.sync.dma_start(rs_r[:], rsum[:])

        qn = 0
        c0 = 0
        for t in range(TILES):
            tc_t = int(nch[t].sum())
            it = idxp.tile([128, TMAX * 8], I16)
            nc.sync.dma_start(it[:, :tc_t * 8], gidx[:, c0 * 8:(c0 + tc_t) * 8])

            A = apool.tile([128, TMAX, 128], MMDT)
            nc.vector.tensor_tensor(
                out=A[:, :tc_t, :],
                in0=io_t[:].unsqueeze(1).to_broadcast([128, tc_t, 128]),
                in1=dst_r[:, c0:c0 + tc_t].unsqueeze(2).to_broadcast(
                    [128, tc_t, 128]),
                op=mybir.AluOpType.is_equal,
            )
            nc.vector.tensor_tensor(
                out=A[:, :tc_t, :],
                in0=A[:, :tc_t, :],
                in1=val_r[:, c0:c0 + tc_t].unsqueeze(2).to_broadcast(
                    [128, tc_t, 128]),
                op=mybir.AluOpType.mult,
            )

            # gathers: one call per <=8 chunks, one R buffer per call
            chunk_src = []  # chunk idx within tile -> (R tile, slot)
            kk = 0
            for g in range(GROUPS):
                n = int(nch[t][g])
                for b0 in range(0, n, 8):
                    nb = min(8, n - b0)
                    R = rpool.tile([128, 8, F], MMDT)
                    nc.gpsimd.dma_gather(
                        out_ap=R[:, :nb, :],
                        in_ap=xdev[WBASE[g]:WBASE[g] + WSZ[g] + 1, :],
                        idxs_ap=it[:, (kk + b0) * 8:(kk + b0 + nb) * 8],
                        num_idxs=nb * 128,
                        num_idxs_reg=nb * 128,
                        elem_size=F,
                        queue_num=qn,
                    )
                    qn = (qn + 1) % 4
                    for j in range(nb):
                        chunk_src.append((R, j))
                kk += n

            pS = psS.tile([128, F], F32)
            for k in range(tc_t):
                R, j = chunk_src[k]
                nc.tensor.matmul(
                    pS[:],
                    lhsT=A[:, k, :],
                    rhs=R[:, j, :],
                    start=(k == 0),
                    stop=(k == tc_t - 1),
                )

            S = spool.tile([128, F], MMDT)
            nc.scalar.copy(S[:], pS[:])
            pT = psT.tile([128, F], MMDT)
            for k in range(4):
                nc.tensor.transpose(
                    pT[:, k * 128:(k + 1) * 128], S[:, k * 128:(k + 1) * 128],
                    id_t[:]
                )
            ST = stpool.tile([128, F], MMDT)
            nc.vector.tensor_copy(ST[:], pT[:])
            pO = psO.tile([128, F], F32)
            for k in range(4):
                nc.tensor.matmul(
                    pO[:],
                    lhsT=ST[:, k * 128:(k + 1) * 128],
                    rhs=w_t[:, k, :],
                    start=(k == 0),
                    stop=False,
                )
            nc.tensor.matmul(
                pO[:],
                lhsT=rs_r[0:1, t * 128:(t + 1) * 128],
                rhs=b_t[0:1, :],
                start=False,
                stop=True,
            )
            O = opool.tile([128, F], MMDT)
            nc.scalar.copy(O[:], pO[:])
            nc.sync.dma_start(out[t * 128:(t + 1) * 128, :], O[:])
            c0 += tc_t

    nc.compile()
    return nc


def kernel(x, g_rows, g_cols, g_vals, weight, b, trace=False):
    x = np.asarray(x, dtype=np.float32)
    weight = np.asarray(weight, dtype=np.float32)
    b = np.asarray(b, dtype=np.float32)

    n_chunks, TC, gidx, gdst, gval, rsum, rowmap = _preprocess(
        g_rows, g_cols, g_vals)

    x_dev = np.zeros((XROWS, F), NPDT)
    for g in range(GROUPS):
        x_dev[WBASE[g]:WBASE[g] + WSZ[g]] = x[GB[g]:GB[g + 1]]
    iota2 = np.broadcast_to(
        np.arange(128, dtype=np.float32)[None, :], (128, 128)
    ).copy()
    ident = np.eye(128, dtype=np.float32)

    nc = _build_program(n_chunks, TC)

    in_maps = []
    for c in range(CORES):
        in_maps.append({
            "xdev": x_dev,
            "gidx": gidx[c],
            "gdst": gdst[c],
            "gval": gval[c].astype(NPDT),
            "wmat": weight.astype(NPDT),
            "bvec": b.reshape(1, F).astype(NPDT),
            "rsum": rsum[c].reshape(1, NPAD).astype(NPDT),
            "iot": iota2,
            "identt": ident.astype(NPDT),
        })

    res = run_bass_kernel_spmd(nc, in_maps, core_ids=list(range(CORES)),
                               trace=trace)
    full = np.zeros((N, F), np.float32)
    for c in range(CORES):
        oc = np.asarray(res.results[c]["out"], dtype=np.float32)
        valid = rowmap[c] >= 0
        full[rowmap[c][valid]] = oc[valid]
    kernel.last_exec_time_ns = res.exec_time_ns
    kernel.last_results = res
    return full


# revision 10
# speedup vs baseline: 1.7516x; 1.0980x over previous
"""HGNN conv kernel for 8 Trainium2 NeuronCores.

Computes out = segment_sum(g_vals * (x @ W + b)[g_cols], g_rows, N)
reordered as out = (G @ x) @ W + rowsum(G) outer b, so that no
cross-core communication is needed: destination rows are sharded
across the 8 cores, x is replicated into every core's DRAM, and each
core gathers the source rows it needs with SWDGE dma_gather.

v2 layout: dest rows are BIN-PACKED (not contiguous) into the
8*98 = 784 tiles of 128 rows so that every (tile, source-window)
bucket lands just under a uniform chunk budget (9,9,8,8 chunks of
128 edges) -> ~3% slot padding instead of ~13%, uniform 1024-index
gather calls, and cross-core-identical loop bounds for the single
SPMD program.  dst/val/rsum live in SBUF for the whole kernel (two
DMAs at startup), gathers run on 4 SWDGE queues with 10 R-buffers of
prefetch depth, A-matrix build stays on the DVE, PSUM->SBUF copies
move to the idle Activation engine, and the output is written bf16.
"""

import os
import sys

import numpy as np

sys.path.insert(0, "/opt/trn_rl_repo")

import concourse.bacc as bacc
import concourse.bass as bass
import concourse.mybir as mybir
import concourse.tile as tile
from concourse.bass_utils import run_bass_kernel_spmd


def _install_ntff_hook():
    """The agent image's antenv lacks axon_hooks; synthesize it so
    run_bass_kernel_spmd(trace=True) can capture NTFF profiles."""
    import types
    if "antenv.axon_hooks" in sys.modules:
        return
    mod = types.ModuleType("antenv.axon_hooks")
    _h = [None]
    mod.set_axon_ntff_profile_hook = lambda h: _h.__setitem__(0, h)
    mod.get_axon_ntff_profile_hook = lambda: _h[0]
    sys.modules["antenv.axon_hooks"] = mod
    import antenv
    antenv.axon_hooks = mod
    from trn_agent_boot.trn_boot import _ntff_profile_via_ctypes
    mod.set_axon_ntff_profile_hook(
        _ntff_profile_via_ctypes("/opt/axon/libaxon_pjrt.so")
    )


_install_ntff_hook()

N = 100000
F = 512
CORES = 8
TILES = 98
NPAD = TILES * 128     # 12544 output rows per core
NBINS = CORES * TILES  # 784
GROUPS = 4
# source-column windows sized proportional to the (9,8,8,8) chunk
# budget so every bucket has ~3.4% slack vs its cap
GB = np.array([0, 27273, 51516, 75758, 100000], dtype=np.int64)
WSZ = np.diff(GB)                       # [27273, 24243, 24242, 24242]
WBASE = np.concatenate([[0], np.cumsum(WSZ + 1)])  # window bases in xdev
XROWS = int(WBASE[-1])                  # 100004 (one zero pad row per window)
CAPS = np.array([9 * 128, 8 * 128, 8 * 128, 8 * 128], dtype=np.int64)

F32 = mybir.dt.float32
BF16 = mybir.dt.bfloat16
I16 = mybir.dt.int16
MMDT = BF16
import ml_dtypes
NPDT = ml_dtypes.bfloat16


def _pack_rows(D):
    """Assign each dest row to one of 784 bins (<=128 rows each) so that
    per-bin per-group edge counts stay under CAPS.  Snake assignment by
    total degree, then local swap repair."""
    totdeg = D.sum(axis=1)
    order = np.argsort(-totdeg, kind="stable")
    i = np.arange(N)
    r, p = i // NBINS, i % NBINS
    binid = np.where(r % 2 == 0, p, NBINS - 1 - p)
    assign = np.empty(N, np.int64)
    assign[order] = binid

    Fb = np.zeros((NBINS, GROUPS), np.int64)
    np.add.at(Fb, assign, D)

    bins_rows = [[] for _ in range(NBINS)]
    for row in order:  # keep degree-desc order inside each bin
        bins_rows[assign[row]].append(row)

    rng = np.random.default_rng(0)
    for _ in range(20000):
        over = Fb - CAPS
        b, g = np.unravel_index(np.argmax(over), over.shape)
        if over[b, g] <= 0:
            break
        # rows in b with highest group-g degree
        rb = bins_rows[b]
        dgs = D[rb, g]
        r1_cands = np.argsort(-dgs)[:4]
        done = False
        # candidate donor bins, least loaded on g first
        for b2 in np.argsort(Fb[:, g])[:48]:
            if b2 == b:
                continue
            rb2 = bins_rows[b2]
            dgs2 = D[rb2, g]
            r2_cands = np.argsort(dgs2)[:4]
            for r1_i in r1_cands:
                r1 = rb[r1_i]
                for r2_i in r2_cands:
                    r2 = rb2[r2_i]
                    d1, d2 = D[r1], D[r2]
                    if d1[g] <= d2[g]:
                        continue
                    nb2 = Fb[b2] + d1 - d2
                    nb = Fb[b] - d1 + d2
                    if (nb2 <= CAPS).all() and \
                            (nb <= np.maximum(Fb[b], CAPS)).all():
                        rb[r1_i] = r2
                        rb2[r2_i] = r1
                        Fb[b] = nb
                        Fb[b2] = nb2
                        done = True
                        break
                if done:
                    break
            if done:
                break
        if not done:
            break  # stuck on the worst bucket; ceil() absorbs the rest
    return bins_rows, Fb


def _preprocess(g_rows, g_cols, g_vals):
    rows = np.asarray(g_rows, dtype=np.int64)
    cols = np.asarray(g_cols, dtype=np.int64)
    vals = np.asarray(g_vals, dtype=np.float32)
    NNZ = rows.shape[0]

    grp = np.searchsorted(GB[1:-1], cols, side="right")
    D = np.bincount(rows * GROUPS + grp, minlength=N * GROUPS).reshape(N, GROUPS)

    bins_rows, Fb = _pack_rows(D)

    # bins -> (tile, core), grouping bins with equal chunk-need profiles
    need = -(-Fb // 128)  # [784, 4]
    binorder = np.lexsort((need[:, 3], need[:, 2], need[:, 1], need[:, 0]))
    core_of_row = np.empty(N, np.int32)
    tile_of_row = np.empty(N, np.int32)
    loc_of_row = np.empty(N, np.int32)
    rowmap = np.full((CORES, NPAD), -1, np.int64)
    n_chunks = np.zeros((TILES, GROUPS), np.int64)
    for idx, b in enumerate(binorder):
        t, c = idx // CORES, idx % CORES
        lst = bins_rows[b]
        core_of_row[lst] = c
        tile_of_row[lst] = t
        loc_of_row[lst] = np.arange(len(lst))
        rowmap[c, t * 128:t * 128 + len(lst)] = lst
        n_chunks[t] = np.maximum(n_chunks[t], need[b])
    n_chunks = np.maximum(n_chunks, 1)

    TC = int(n_chunks.sum())
    ncf = n_chunks.reshape(-1)                      # [TILES*GROUPS]
    col_off = np.zeros(TILES * GROUPS + 1, np.int64)
    np.cumsum(ncf, out=col_off[1:])
    slot_off = col_off * 128
    SLOTS = TC * 128

    ec = core_of_row[rows]
    et = tile_of_row[rows]
    el = loc_of_row[rows]
    sloc = (cols - GB[grp]).astype(np.int16)

    key = (((ec.astype(np.int64) * TILES + et) * GROUPS + grp) * (2**18)
           + cols - GB[grp])
    order = np.argsort(key, kind="stable")

    core_cnt = np.bincount(ec, minlength=CORES)
    core_start = np.zeros(CORES + 1, np.int64)
    np.cumsum(core_cnt, out=core_start[1:])

    # default pad index per slot = its window's zero pad row
    wsz_tg = np.tile(WSZ, TILES)                    # [TILES*GROUPS]
    pad_flat = np.repeat(wsz_tg, ncf * 128).astype(np.int16)

    gidx = np.empty((CORES, 128, TC * 8), np.int16)
    gdst = np.empty((CORES, 128, TC), np.float32)
    gval = np.empty((CORES, 128, TC), np.float32)
    rsum = np.zeros((CORES, NPAD), np.float32)

    for c in range(CORES):
        seg = order[core_start[c]:core_start[c + 1]]
        tg = et[seg].astype(np.int64) * GROUPS + grp[seg]   # non-decreasing
        cnt_tg = np.bincount(tg, minlength=TILES * GROUPS)
        assert (cnt_tg <= ncf * 128).all()
        gstart = np.zeros(TILES * GROUPS, np.int64)
        np.cumsum(cnt_tg[:-1], out=gstart[1:])
        pos = np.arange(len(seg), dtype=np.int64) - np.repeat(gstart, cnt_tg)
        slot = slot_off[tg] + pos

        idx_flat = pad_flat.copy()
        idx_flat[slot] = sloc[seg]
        d_flat = np.zeros(SLOTS, np.float32)
        d_flat[slot] = el[seg].astype(np.float32)
        v_flat = np.zeros(SLOTS, np.float32)
        v_flat[slot] = vals[seg]

        gdst[c] = d_flat.reshape(TC, 128).T
        gval[c] = v_flat.reshape(TC, 128).T
        # idx wrap: within each (t,g) bucket, idx j -> [j%16, j//16], x8 replicated
        for tg_i in range(TILES * GROUPS):
            n = ncf[tg_i]
            a = slot_off[tg_i]
            bcol = col_off[tg_i] * 8
            blk = idx_flat[a:a + n * 128].reshape(n * 8, 16).T
            gidx[c][:, bcol:bcol + n * 8] = np.tile(blk, (8, 1))

        rs = np.bincount(et[seg] * 128 + el[seg],
                         weights=vals[seg].astype(np.float64), minlength=NPAD)
        rsum[c] = rs.astype(np.float32)

    return n_chunks, TC, gidx, gdst, gval, rsum, rowmap


def _build_program(n_chunks, TC):
    nch = n_chunks  # [TILES, GROUPS]
    TMAX = int(nch.sum(axis=1).max())

    nc = bacc.Bacc(
        "TRN2",
        target_bir_lowering=False,
        debug=False,
        enable_asserts=False,
        num_devices=CORES,
        num_swdge_queues=4,
    )
    xdev = nc.dram_tensor("xdev", [XROWS, F], MMDT, kind="ExternalInput").ap()
    gidx = nc.dram_tensor("gidx", [128, TC * 8], I16, kind="ExternalInput").ap()
    gdst = nc.dram_tensor("gdst", [128, TC], F32, kind="ExternalInput").ap()
    gval = nc.dram_tensor("gval", [128, TC], MMDT, kind="ExternalInput").ap()
    wmat = nc.dram_tensor("wmat", [F, F], MMDT, kind="ExternalInput").ap()
    bvec = nc.dram_tensor("bvec", [1, F], MMDT, kind="ExternalInput").ap()
    rsum = nc.dram_tensor("rsum", [1, NPAD], MMDT, kind="ExternalInput").ap()
    iot = nc.dram_tensor("iot", [128, 128], F32, kind="ExternalInput").ap()
    identt = nc.dram_tensor("identt", [128, 128], MMDT, kind="ExternalInput").ap()
    out = nc.dram_tensor("out", [NPAD, F], MMDT, kind="ExternalOutput").ap()

    from contextlib import ExitStack

    with tile.TileContext(nc) as tc, ExitStack() as ctx:
        cpool = ctx.enter_context(tc.tile_pool(name="const", bufs=1))
        idxp = ctx.enter_context(tc.tile_pool(name="idxp", bufs=6))
        apool = ctx.enter_context(tc.tile_pool(name="ap", bufs=2))
        rpool = ctx.enter_context(tc.tile_pool(name="rp", bufs=8))
        spool = ctx.enter_context(tc.tile_pool(name="sp", bufs=2))
        stpool = ctx.enter_context(tc.tile_pool(name="stp", bufs=2))
        opool = ctx.enter_context(tc.tile_pool(name="op", bufs=2))
        psS = ctx.enter_context(tc.tile_pool(name="psS", bufs=2, space="PSUM"))
        psT = ctx.enter_context(tc.tile_pool(name="psT", bufs=2, space="PSUM"))
        psO = ctx.enter_context(tc.tile_pool(name="psO", bufs=2, space="PSUM"))

        # static SBUF residents
        w_t = cpool.tile([128, 4, F], MMDT)
        for k in range(4):
            nc.sync.dma_start(w_t[:, k, :], wmat[k * 128:(k + 1) * 128, :])
        b_t = cpool.tile([1, F], MMDT)
        nc.sync.dma_start(b_t[:], bvec[:])
        io_t = cpool.tile([128, 128], F32)
        nc.sync.dma_start(io_t[:], iot[:])
        id_t = cpool.tile([128, 128], MMDT)
        nc.sync.dma_start(id_t[:], identt[:])
        dst_r = cpool.tile([128, TC], F32)
        nc.sync.dma_start(dst_r[:], gdst[:])
        val_r = cpool.tile([128, TC], MMDT)
        nc.sync.dma_start(val_r[:], gval[:])
        rs_r = cpool.tile([1, NPAD], MMDT)
        nc.sync.dma_start(rs_r[:], rsum[:])

        qn = 0
        c0 = 0
        for t in range(TILES):
            tc_t = int(nch[t].sum())
            it = idxp.tile([128, TMAX * 8], I16)
            nc.sync.dma_start(it[:, :tc_t * 8], gidx[:, c0 * 8:(c0 + tc_t) * 8])

            A = apool.tile([128, TMAX, 128], MMDT)
            nc.vector.tensor_tensor(
                out=A[:, :tc_t, :],
                in0=io_t[:].unsqueeze(1).to_broadcast([128, tc_t, 128]),
                in1=dst_r[:, c0:c0 + tc_t].unsqueeze(2).to_broadcast(
                    [128, tc_t, 128]),
                op=mybir.AluOpType.is_equal,
            )
            nc.vector.tensor_tensor(
                out=A[:, :tc_t, :],
                in0=A[:, :tc_t, :],
                in1=val_r[:, c0:c0 + tc_t].unsqueeze(2).to_broadcast(
                    [128, tc_t, 128]),
                op=mybir.AluOpType.mult,
            )

            # gathers: one call per (tile, group), one R buffer per call
            chunk_src = []  # chunk idx within tile -> (R tile, slot)
            kk = 0
            for g in range(GROUPS):
                n = int(nch[t][g])
                R = rpool.tile([128, 9, F], MMDT)
                # single_packet caps one call at 1024 indices (8 chunks)
                for b0 in range(0, n, 8):
                    nb = min(8, n - b0)
                    nc.gpsimd.dma_gather(
                        out_ap=R[:, b0:b0 + nb, :],
                        in_ap=xdev[WBASE[g]:WBASE[g] + WSZ[g] + 1, :],
                        idxs_ap=it[:, (kk + b0) * 8:(kk + b0 + nb) * 8],
                        num_idxs=nb * 128,
                        num_idxs_reg=nb * 128,
                        elem_size=F,
                        queue_num=qn,
                    )
                    qn = (qn + 1) % 4
                for j in range(n):
                    chunk_src.append((R, j))
                kk += n

            pS = psS.tile([128, F], F32)
            for k in range(tc_t):
                R, j = chunk_src[k]
                nc.tensor.matmul(
                    pS[:],
                    lhsT=A[:, k, :],
                    rhs=R[:, j, :],
                    start=(k == 0),
                    stop=(k == tc_t - 1),
                )

            S = spool.tile([128, F], MMDT)
            nc.scalar.copy(S[:], pS[:])
            pT = psT.tile([128, F], MMDT)
            for k in range(4):
                nc.tensor.transpose(
                    pT[:, k * 128:(k + 1) * 128], S[:, k * 128:(k + 1) * 128],
                    id_t[:]
                )
            ST = stpool.tile([128, F], MMDT)
            nc.vector.tensor_copy(ST[:], pT[:])
            pO = psO.tile([128, F], F32)
            for k in range(4):
                nc.tensor.matmul(
                    pO[:],
                    lhsT=ST[:, k * 128:(k + 1) * 128],
                    rhs=w_t[:, k, :],
                    start=(k == 0),
                    stop=False,
                )
            nc.tensor.matmul(
                pO[:],
                lhsT=rs_r[0:1, t * 128:(t + 1) * 128],
                rhs=b_t[0:1, :],
                start=False,
                stop=True,
            )
            O = opool.tile([128, F], MMDT)
            nc.scalar.copy(O[:], pO[:])
            nc.sync.dma_start(out[t * 128:(t + 1) * 128, :], O[:])
            c0 += tc_t

    nc.compile()
    return nc


def kernel(x, g_rows, g_cols, g_vals, weight, b, trace=False):
    x = np.asarray(x, dtype=np.float32)
    weight = np.asarray(weight, dtype=np.float32)
    b = np.asarray(b, dtype=np.float32)

    n_chunks, TC, gidx, gdst, gval, rsum, rowmap = _preprocess(
        g_rows, g_cols, g_vals)

    x_dev = np.zeros((XROWS, F), NPDT)
    for g in range(GROUPS):
        x_dev[WBASE[g]:WBASE[g] + WSZ[g]] = x[GB[g]:GB[g + 1]]
    iota2 = np.broadcast_to(
        np.arange(128, dtype=np.float32)[None, :], (128, 128)
    ).copy()
    ident = np.eye(128, dtype=np.float32)

    nc = _build_program(n_chunks, TC)

    in_maps = []
    for c in range(CORES):
        in_maps.append({
            "xdev": x_dev,
            "gidx": gidx[c],
            "gdst": gdst[c],
            "gval": gval[c].astype(NPDT),
            "wmat": weight.astype(NPDT),
            "bvec": b.reshape(1, F).astype(NPDT),
            "rsum": rsum[c].reshape(1, NPAD).astype(NPDT),
            "iot": iota2,
            "identt": ident.astype(NPDT),
        })

    res = run_bass_kernel_spmd(nc, in_maps, core_ids=list(range(CORES)),
                               trace=trace)
    full = np.zeros((N, F), np.float32)
    for c in range(CORES):
        oc = np.asarray(res.results[c]["out"], dtype=np.float32)
        valid = rowmap[c] >= 0
        full[rowmap[c][valid]] = oc[valid]
    kernel.last_exec_time_ns = res.exec_time_ns
    kernel.last_results = res
    return full
